# revision 22
# baseline (speedup 1.0000x reference)
"""Trainium2 Bass kernel for a 4-layer LSTM-style stack with local+global logits.

Computation (per example row x of the [16384, 512] input):
    h0 = 0, c0 = 0
    for i in 1..4:
        z  = [x, h_{i-1}] @ W{f,i,o,c} + b        (4 gates, K = 1024)
        c  = tanh(z_c) * sigmoid(z_i) + sigmoid(z_f) * c
        h  = sigmoid(z_o) * tanh(c)
        local_i = h @ Wl_i + bl_i
    global = [x, h4] @ Wg + bg
Returns (concat(local_1..4) [16384, 960], global [16384, 960]).

Strategy (v4):
  - Data-parallel over 8 cores: 2048 rows each, weights replicated.
  - Z = x @ W_top + b computed once per example (bf16), stored scaled x128
    as one [128, 16, 512] tile per quarter (column order t-major: of = t*4+g
    so each hid-tile t's four gates are contiguous).
  - Layers 2-4 hidden-state matmuls run in fp8 e4m3 with
    perf_mode=DoubleRow (K=256 per instruction, 2 fp8 weights per PE cell,
    ~2x bf16 FLOP rate, measured 216 ns per K=256/FD=512 matmul).
    W_bot is prescaled x128 into fp8; h is cast to fp8 unscaled.
    Gates = act((ps + z128)/128) via the activation input scale.
  - Gate phases are Vector-engine-bound (PSUM+z adds); PE work from other
    pipeline stages (Z of later quarters, locals, globals) is emitted
    BETWEEN gate t-blocks so the PSUM ring rotates filler matmuls into the
    windows where the PE would otherwise stall on PSUM drain.
  - Logits stay bf16; eviction via Scalar-engine Copy to bf16 outputs
    (harness-compared at 2e-2 rel-l2; bias added on host, zero here).
  - Elementwise/activation ops are fused across hid-tiles (strided APs).
"""

import os
import sys

import numpy as np

for _p in ("/opt/trn_rl_repo", "/root/.axon_site/_ro/trn_rl_repo"):
    if os.path.isdir(_p) and _p not in sys.path:
        sys.path.insert(0, _p)

import ml_dtypes

import concourse.bass as bass
import concourse.tile as tile
from concourse import bacc, mybir
from concourse.bass_utils import run_bass_kernel_spmd

BF16 = mybir.dt.bfloat16
F32 = mybir.dt.float32
FP8 = mybir.dt.float8e4
AF = mybir.ActivationFunctionType
ALU = mybir.AluOpType
DR = mybir.MatmulPerfMode.DoubleRow

N_CORES = 8
N = 16384
K = 512                  # input features
U = 512                  # hidden units
MC = N // N_CORES        # 2048 rows per core
NQ = 4                   # quarters per core
EXQ = MC // NQ           # 512 examples per quarter
NCLS = [64, 128, 256, 512]
OFFS = [0, 64, 192, 448]
TOT = 960
WS = 128.0               # fp8 weight prescale and z storage scale

IDZ_LAYERS = set()       # layers whose Z-add streams through the PE

LAST_RESULT = None       # BassKernelResults of the most recent run (for test.py)


def _build_program():
    nc = bacc.Bacc("TRN2", target_bir_lowering=False, debug=False)

    xt_d = nc.dram_tensor("xt", [K, MC], BF16, kind="ExternalInput")
    wtop_d = nc.dram_tensor("wtop", [K, 4 * U], BF16, kind="ExternalInput")
    w8a_d = nc.dram_tensor("w8a", [128, 2, 4 * U], FP8, kind="ExternalInput")
    w8b_d = nc.dram_tensor("w8b", [128, 2, 4 * U], FP8, kind="ExternalInput")
    wl_d = nc.dram_tensor("wl", [U, TOT], BF16, kind="ExternalInput")
    wg_d = nc.dram_tensor("wg", [K + U, TOT], BF16, kind="ExternalInput")
    bg128_d = nc.dram_tensor("bg128", [128, 16], F32, kind="ExternalInput")
    ident_d = nc.dram_tensor("ident", [128, 128], BF16, kind="ExternalInput")
    oloc_d = nc.dram_tensor("oloc", [MC, TOT], BF16, kind="ExternalOutput")
    oglb_d = nc.dram_tensor("oglb", [MC, TOT], BF16, kind="ExternalOutput")

    with tile.TileContext(nc) as tc:
        with (
            tc.tile_pool(name="wpool", bufs=1) as wpool,
            tc.tile_pool(name="xpool", bufs=4) as xpool,
            tc.tile_pool(name="zpool", bufs=3) as zpool,
            tc.tile_pool(name="prepool", bufs=1) as prepool,
            tc.tile_pool(name="hpool", bufs=2) as hpool,
            tc.tile_pool(name="cpool", bufs=2) as cpool,
            tc.tile_pool(name="h8pool", bufs=2) as h8pool,
            tc.tile_pool(name="ttp", bufs=1) as ttp,
            tc.tile_pool(name="tcp", bufs=1) as tcp,
            tc.tile_pool(name="l1p", bufs=1) as l1p,
            tc.tile_pool(name="lop", bufs=1) as lop,
            tc.tile_pool(name="glop", bufs=1) as glop,
            tc.tile_pool(name="gpsum", bufs=2, space="PSUM") as gpsum,
        ):
            # ---- resident weights (DMA order = first-use order) ---------
            xs_pre = {}
            tiles = []
            wtop_sb = []
            for kt in range(4):
                t = xpool.tile([128, EXQ], BF16, tag=f"x{kt}")
                nc.sync.dma_start(t[:], xt_d[kt * 128:(kt + 1) * 128, 0:EXQ])
                tiles.append(t)
                w = wpool.tile([128, 4 * U], BF16, tag=f"wt{kt}")
                nc.sync.dma_start(w[:], wtop_d[kt * 128:(kt + 1) * 128, :])
                wtop_sb.append(w)
            xs_pre[0] = tiles
            bg128_sb = wpool.tile([128, 16], F32, tag="bg128")
            nc.sync.dma_start(bg128_sb[:], bg128_d[:])
            tiles = []
            for kt in range(4):
                t = xpool.tile([128, EXQ], BF16, tag=f"x{kt}")
                nc.sync.dma_start(
                    t[:], xt_d[kt * 128:(kt + 1) * 128, EXQ:2 * EXQ])
                tiles.append(t)
            xs_pre[1] = tiles
            ident_sb = wpool.tile([128, 128], BF16, tag="ident")
            nc.sync.dma_start(ident_sb[:], ident_d[:])
            w8_sb = []
            for j, d in enumerate((w8a_d, w8b_d)):
                t = wpool.tile([128, 2, 4 * U], FP8, tag=f"w8{j}")
                nc.sync.dma_start(t[:], d[:])
                w8_sb.append(t)
            wl_sb = []
            for kt in range(4):
                t = wpool.tile([128, TOT], BF16, tag=f"wl{kt}")
                nc.sync.dma_start(t[:], wl_d[kt * 128:(kt + 1) * 128, :])
                wl_sb.append(t)
            wg_sb = []
            for kt in range(8):
                t = wpool.tile([128, TOT], BF16, tag=f"wg{kt}")
                nc.sync.dma_start(t[:], wg_d[kt * 128:(kt + 1) * 128, :])
                wg_sb.append(t)

            # per-quarter live state
            xs = [None] * NQ     # 4 x [128, EXQ] bf16 X^T tiles
            zs = [None] * NQ     # [128, 16, 512] bf16 tile: z*128, of = t*4+g
            hs = [None] * NQ     # 2 x [128, 2, EXQ] bf16 (j-pairs)
            cs = [None] * NQ     # 2 x [128, 2, EXQ] bf16
            h8s = [None] * NQ    # 2 x [128, 2, EXQ] fp8

            def stage_x(q):
                """DMA this quarter's x tiles."""
                if q in xs_pre:
                    xs[q] = xs_pre.pop(q)
                else:
                    xs[q] = []
                    for kt in range(4):
                        t = xpool.tile([128, EXQ], BF16, tag=f"x{kt}")
                        nc.sync.dma_start(
                            t[:], xt_d[kt * 128:(kt + 1) * 128,
                                       q * EXQ:(q + 1) * EXQ])
                        xs[q].append(t)

            def z_chunk(q, og):
                """One of-group (4 of-tiles) of Z: matmul + fused evict.
                z128 = 128*(x @ W_top) [+ 128*b per-of when bias nonzero]."""
                zq = zs[q]
                ps = gpsum.tile([128, 4, EXQ], F32, tag="gp4", name="ps")
                for i in range(4):
                    of = og * 4 + i
                    for kt in range(4):
                        nc.tensor.matmul(
                            ps[:, i, :],
                            wtop_sb[kt][:, of * 128:(of + 1) * 128],
                            xs[q][kt][:], start=(kt == 0), stop=(kt == 3))
                nc.vector.tensor_scalar(
                    zq[:, og * 4:(og + 1) * 4, :], ps[:], WS, None, ALU.mult)

            def stage_z_alloc(q):
                zq = zpool.tile([128, 16, EXQ], BF16, tag="z", name="z")
                zs[q] = zq

            def cand_block(q, pre, j, first, want_h8=False):
                """c/h/h8 update for hid-pair j from gate tile `pre`."""
                a = q % 2
                b0 = 8 * j
                f_ap = pre[:, b0 + 0:b0 + 8:4, :]
                ig_ap = pre[:, b0 + 1:b0 + 8:4, :]
                og_ap = pre[:, b0 + 2:b0 + 8:4, :]
                ch_ap = pre[:, b0 + 3:b0 + 8:4, :]
                cn = cpool.tile([128, 2, EXQ], BF16, tag=f"c{a}{j}",
                                name="cn")
                if first:
                    nc.vector.tensor_mul(cn[:], ig_ap, ch_ap)
                else:
                    t1 = ttp.tile([128, 2, EXQ], BF16, tag="t1", name="t1")
                    nc.vector.tensor_mul(t1[:], ig_ap, ch_ap)
                    t2 = ttp.tile([128, 2, EXQ], BF16, tag="t2", name="t2")
                    nc.vector.tensor_mul(t2[:], f_ap, cs[q][j][:])
                    nc.vector.tensor_add(cn[:], t1[:], t2[:])
                cs[q][j] = cn
                tc_t = tcp.tile([128, 2, EXQ], BF16, tag="tc", name="tc")
                nc.scalar.activation(tc_t[:], cn[:], AF.Tanh)
                if want_h8:
                    h8 = h8pool.tile([128, 2, EXQ], FP8,
                                     tag=f"h8{q % 2}{j}", name="h8")
                    nc.vector.tensor_mul(h8[:], og_ap, tc_t[:])
                    h8s[q][j] = h8
                hn = hpool.tile([128, 2, EXQ], BF16, tag=f"h{a}{j}",
                                name="hn")
                nc.vector.tensor_mul(hn[:], og_ap, tc_t[:])
                hs[q][j] = hn
                return hn

            def cand_half(q, pre, t, half, state, want_h8):
                """Single-t (half-pair) cand chain: shortens the exposed
                dependency tail at layer boundaries.  half 0 allocates the
                j=1 c/h/h8 tiles; half 1 publishes them."""
                a = q % 2
                j = t // 2
                f_ap = pre[:, t * 4 + 0, :]
                ig_ap = pre[:, t * 4 + 1, :]
                og_ap = pre[:, t * 4 + 2, :]
                ch_ap = pre[:, t * 4 + 3, :]
                if half == 0:
                    cn = cpool.tile([128, 2, EXQ], BF16, tag=f"c{a}{j}",
                                    name="cn")
                    hn = hpool.tile([128, 2, EXQ], BF16, tag=f"h{a}{j}",
                                    name="hn")
                    h8 = None
                    if want_h8:
                        h8 = h8pool.tile([128, 2, EXQ], FP8,
                                         tag=f"h8{a}{j}", name="h8")
                    state[q] = (cn, hn, h8)
                cn, hn, h8 = state[q]
                t1 = ttp.tile([128, EXQ], BF16, tag="t1h", name="t1")
                nc.vector.tensor_mul(t1[:], ig_ap, ch_ap)
                t2 = ttp.tile([128, EXQ], BF16, tag="t2h", name="t2")
                nc.vector.tensor_mul(t2[:], f_ap, cs[q][j][:, half, :])
                nc.vector.tensor_add(cn[:, half, :], t1[:], t2[:])
                tc_t = tcp.tile([128, EXQ], BF16, tag="tch", name="tc")
                nc.scalar.activation(tc_t[:], cn[:, half, :], AF.Tanh)
                if h8 is not None:
                    nc.vector.tensor_mul(h8[:, half, :], og_ap, tc_t[:])
                nc.vector.tensor_mul(hn[:, half, :], og_ap, tc_t[:])
                if half == 1:
                    cs[q][j] = cn
                    hs[q][j] = hn
                    if h8 is not None:
                        h8s[q][j] = h8

            def stage_l1(q):
                """Layer 1: h0 = 0, gates straight from z (no f gate)."""
                hs[q] = [None, None]
                cs[q] = [None, None]
                h8s[q] = [None, None]
                zq = zs[q]
                for j in range(2):
                    b0 = 8 * j
                    gi = l1p.tile([128, 2, EXQ], BF16, tag="gi", name="gi")
                    nc.scalar.activation(
                        gi[:], zq[:, b0 + 1:b0 + 8:4, :], AF.Sigmoid,
                        scale=1.0 / WS)
                    go = l1p.tile([128, 2, EXQ], BF16, tag="go", name="go")
                    nc.scalar.activation(
                        go[:], zq[:, b0 + 2:b0 + 8:4, :], AF.Sigmoid,
                        scale=1.0 / WS)
                    ch = l1p.tile([128, 2, EXQ], BF16, tag="ch", name="ch")
                    nc.scalar.activation(
                        ch[:], zq[:, b0 + 3:b0 + 8:4, :], AF.Tanh,
                        scale=1.0 / WS)
                    a = q % 2
                    cn = cpool.tile([128, 2, EXQ], BF16, tag=f"c{a}{j}",
                                    name="cn")
                    nc.vector.tensor_mul(cn[:], gi[:], ch[:])
                    cs[q][j] = cn
                    tc_t = tcp.tile([128, 2, EXQ], BF16, tag="tc", name="tc")
                    nc.scalar.activation(tc_t[:], cn[:], AF.Tanh)
                    h8 = h8pool.tile([128, 2, EXQ], FP8,
                                     tag=f"h8{q % 2}{j}", name="h8")
                    nc.vector.tensor_mul(h8[:], go[:], tc_t[:])
                    h8s[q][j] = h8
                    hn = hpool.tile([128, 2, EXQ], BF16, tag=f"h{a}{j}",
                                    name="hn")
                    nc.vector.tensor_mul(hn[:], go[:], tc_t[:])
                    hs[q][j] = hn

            def h_slice(h_prev, kt, e):
                j, i = kt // 2, kt % 2
                return h_prev[j][:, i, e * 128:(e + 1) * 128]

            def locals_chunk(q, layer, h_prev):
                """local_i = h_i @ Wl_i (+bias on host), natural layout."""
                off, ncl = OFFS[layer], NCLS[layer]
                ps = gpsum.tile([128, 4, EXQ], F32, tag="gp4", name="ps")
                for e in range(4):
                    for kt in range(4):
                        nc.tensor.matmul(
                            ps[:, e, 0:ncl],
                            h_slice(h_prev, kt, e),
                            wl_sb[kt][:, off:off + ncl],
                            start=(kt == 0), stop=(kt == 3))
                lo = lop.tile([128, 4, EXQ], BF16, tag="lo", name="lo")
                nc.scalar.activation(
                    lo[:, :, 0:ncl], ps[:, :, 0:ncl], AF.Copy)
                for e in range(4):
                    r0 = q * EXQ + e * 128
                    nc.sync.dma_start(
                        oloc_d[r0:r0 + 128, off:off + ncl], lo[:, e, 0:ncl])

            def global_chunk(q, p, h_prev):
                """globals for examples pair p (2 e-tiles) of quarter q."""
                ps = gpsum.tile([128, 4, EXQ], F32, tag="gp4", name="ps")
                for ie in range(2):
                    e = 2 * p + ie
                    for s, w in ((0, 512), (1, 448)):
                        out = ps[:, ie * 2 + s, 0:w]
                        for kt in range(8):
                            if kt < 4:
                                st = xs[q][kt][:, e * 128:(e + 1) * 128]
                            else:
                                st = h_slice(h_prev, kt - 4, e)
                            nc.tensor.matmul(
                                out, st,
                                wg_sb[kt][:, s * 512:s * 512 + w],
                                start=(kt == 0), stop=(kt == 7))
                gt = glop.tile([128, 4, EXQ], BF16, tag="glo", name="gt")
                nc.scalar.activation(gt[:], ps[:], AF.Copy)
                for ie in range(2):
                    e = 2 * p + ie
                    r0 = q * EXQ + e * 128
                    nc.sync.dma_start(
                        oglb_d[r0:r0 + 128, 0:512], gt[:, ie * 2, :])
                    nc.sync.dma_start(
                        oglb_d[r0:r0 + 128, 512:960], gt[:, ie * 2 + 1, 0:448])

            def stage_pair(qa, qb, layer, fillers=()):
                """One gate layer (2..4) for two quarters, interleaved, with
                PE filler chunks emitted between t-blocks."""
                fillers = list(fillers)
                idz = layer in IDZ_LAYERS
                h8a, h8b = h8s[qa], h8s[qb]
                hs[qa] = [None, None]
                hs[qb] = [None, None]
                h8s[qa] = [None, None]
                h8s[qb] = [None, None]
                pres = {}
                half_state = {}
                for q in (qa, qb):
                    pres[q] = prepool.tile(
                        [128, 16, EXQ], BF16, tag=f"pre{q % 2}",
                        name=f"pre{q % 2}")
                for t in range(4):
                    pss = {}
                    for q in (qa, qb):
                        pss[q] = gpsum.tile(
                            [128, 4, EXQ], F32, tag="gp4", name="ps")
                    if idz:
                        for q in (qa, qb):
                            for g in range(4):
                                nc.tensor.matmul(
                                    pss[q][:, g, :], ident_sb[:],
                                    zs[q][:, t * 4 + g, :],
                                    start=True, stop=False)
                    # j=0 matmuls for every gate first: h8[j1] of the
                    # previous layer is produced late by its last cand
                    # chain, and the in-order PE queue must not block on it
                    for j in range(2):
                        for g in range(4):
                            w8ap = w8_sb[j][:, :, (t * 4 + g) * 128:
                                            (t * 4 + g + 1) * 128]
                            for q, h8p in ((qa, h8a), (qb, h8b)):
                                nc.tensor.matmul(
                                    pss[q][:, g, :], w8ap, h8p[j][:],
                                    start=(j == 0 and not idz),
                                    stop=(j == 1), perf_mode=DR)
                    for q in (qa, qb):
                        ps, pre = pss[q], pres[q]
                        nc.vector.tensor_tensor(
                            pre[:, t * 4:(t + 1) * 4, :], ps[:],
                            zs[q][:, t * 4:(t + 1) * 4, :], ALU.add)
                        if t == 3:
                            # split activations so the final cand chain
                            # (the exposed layer-boundary tail) starts
                            # as early as possible
                            nc.scalar.activation(
                                pre[:, t * 4 + 3, :], pre[:, t * 4 + 3, :],
                                AF.Tanh, scale=1.0 / WS)
                            nc.scalar.activation(
                                pre[:, t * 4 + 1, :], pre[:, t * 4 + 1, :],
                                AF.Sigmoid, scale=1.0 / WS)
                            nc.scalar.activation(
                                pre[:, t * 4:t * 4 + 3:2, :],
                                pre[:, t * 4:t * 4 + 3:2, :],
                                AF.Sigmoid, scale=1.0 / WS)
                        else:
                            nc.scalar.activation(
                                pre[:, t * 4:t * 4 + 3, :],
                                pre[:, t * 4:t * 4 + 3, :],
                                AF.Sigmoid, scale=1.0 / WS)
                            nc.scalar.activation(
                                pre[:, t * 4 + 3, :], pre[:, t * 4 + 3, :],
                                AF.Tanh, scale=1.0 / WS)
                    if t == 1:
                        for q in (qa, qb):
                            cand_block(q, pres[q], 0, first=False,
                                       want_h8=(layer < 4))
                    elif t >= 2:
                        for q in (qa, qb):
                            cand_half(q, pres[q], t, t - 2, half_state,
                                      want_h8=(layer < 4))
                    # inject one PE filler chunk per t-block
                    if fillers:
                        fillers.pop(0)()
                for f in fillers:
                    f()

            # ---- schedule ----------------------------------------------
            # Pipeline: pair (0,1) gate layers carry Z(2)/Z(3)/locals as
            # PE fillers; pair (2,3) gate layers carry pair-A globals.
            stage_x(0)
            stage_x(1)
            stage_z_alloc(0)
            stage_z_alloc(1)
            for og in range(4):
                z_chunk(0, og)
            for og in range(4):
                z_chunk(1, og)
            stage_l1(0)
            stage_x(2)
            stage_z_alloc(2)
            z_chunk(2, 0)
            z_chunk(2, 1)
            stage_l1(1)
            h1 = {q: hs[q] for q in (0, 1)}
            stage_pair(0, 1, 2, fillers=[
                lambda: z_chunk(2, 2),
                lambda: z_chunk(2, 3),
                lambda: locals_chunk(0, 0, h1[0]),
                lambda: locals_chunk(1, 0, h1[1]),
            ])
            stage_x(3)
            h2 = {q: hs[q] for q in (0, 1)}
            stage_pair(0, 1, 3, fillers=[
                lambda: locals_chunk(0, 1, h2[0]),
                lambda: locals_chunk(1, 1, h2[1]),
            ])
            h3 = {q: hs[q] for q in (0, 1)}
            stage_pair(0, 1, 4, fillers=[
                lambda: locals_chunk(0, 2, h3[0]),
                lambda: locals_chunk(1, 2, h3[1]),
            ])
            # z(3) lands in z(0)'s slot: safe only after P4's pre-adds
            stage_z_alloc(3)
            for og in range(4):
                z_chunk(3, og)
            h4 = {q: hs[q] for q in (0, 1)}
            locals_chunk(0, 3, h4[0])
            stage_l1(2)
            global_chunk(0, 0, h4[0])
            global_chunk(0, 1, h4[0])
            locals_chunk(1, 3, h4[1])
            stage_l1(3)
            global_chunk(1, 0, h4[1])
            global_chunk(1, 1, h4[1])
            h1b = {q: hs[q] for q in (2, 3)}
            stage_pair(2, 3, 2, fillers=[
                lambda: locals_chunk(2, 0, h1b[2]),
                lambda: locals_chunk(3, 0, h1b[3]),
            ])
            h2b = {q: hs[q] for q in (2, 3)}
            stage_pair(2, 3, 3, fillers=[
                lambda: locals_chunk(2, 1, h2b[2]),
                lambda: locals_chunk(3, 1, h2b[3]),
            ])
            h3b = {q: hs[q] for q in (2, 3)}
            stage_pair(2, 3, 4, fillers=[
                lambda: locals_chunk(2, 2, h3b[2]),
                lambda: locals_chunk(3, 2, h3b[3]),
            ])
            for q in (2, 3):
                locals_chunk(q, 3, hs[q])
                global_chunk(q, 0, hs[q])
                global_chunk(q, 1, hs[q])

    nc.compile()
    return nc


_PROGRAM = None


def _get_program():
    global _PROGRAM
    if _PROGRAM is None:
        _PROGRAM = _build_program()
    return _PROGRAM


def kernel(inputs, Wf, bf, Wi, bi, Wo, bo, Wc, bc,
           Wl0, bl0, Wl1, bl1, Wl2, bl2, Wl3, bl3, Wg, bg):
    global LAST_RESULT
    bf16 = ml_dtypes.bfloat16
    fp8 = ml_dtypes.float8_e4m3

    inputs = np.ascontiguousarray(np.asarray(inputs, dtype=np.float32))
    xt_all = inputs.T.astype(bf16)                    # [512, 16384]
    wcat = np.concatenate(
        [np.asarray(w, np.float32) for w in (Wf, Wi, Wo, Wc)], axis=1)
    bcat = np.concatenate(
        [np.asarray(b, np.float32) for b in (bf, bi, bo, bc)])  # [2048]
    # gate biases are zero in this problem; the on-device Z path assumes so
    # (they would otherwise need the per-of bias variant of the Z eviction)
    assert not np.any(bcat), "nonzero gate biases unsupported by this kernel"

    # t-major column permutation: new col (t*4+g)*128+m <- old g*512+t*128+m
    P = np.empty(2048, np.int64)
    for t in range(4):
        for g in range(4):
            P[(t * 4 + g) * 128:(t * 4 + g + 1) * 128] = np.arange(
                g * 512 + t * 128, g * 512 + t * 128 + 128)
    wcat_p = wcat[:, P]
    bcat_p = bcat[P]

    wtop = np.ascontiguousarray(wcat_p[:512]).astype(bf16)      # [512, 2048]
    wbot = wcat_p[512:]                                          # [512, 2048]
    w8 = []
    for j in range(2):
        a = np.empty((128, 2, 2048), np.float32)
        for i in range(2):
            a[:, i, :] = wbot[128 * (2 * j + i):128 * (2 * j + i + 1), :]
        w8.append(np.ascontiguousarray((a * WS).astype(fp8)))
    bg128 = np.ascontiguousarray(
        (WS * bcat_p).reshape(16, 128).T.astype(np.float32))     # [128, 16]
    wl = np.concatenate(
        [np.asarray(w, np.float32) for w in (Wl0, Wl1, Wl2, Wl3)],
        axis=1).astype(bf16)                          # [512, 960]
    wg = np.asarray(Wg, np.float32).astype(bf16)      # [1024, 960]
    ident = np.eye(128, dtype=np.float32).astype(bf16)

    in_maps = []
    for c in range(N_CORES):
        in_maps.append({
            "xt": np.ascontiguousarray(xt_all[:, c * MC:(c + 1) * MC]),
            "wtop": wtop, "w8a": w8[0], "w8b": w8[1],
            "wl": wl, "wg": wg, "bg128": bg128, "ident": ident,
        })

    nc = _get_program()
    trace = os.environ.get("BASS_KERNEL_TRACE", "0") == "1"
    tmpdir = os.environ.get("BASS_KERNEL_TMPDIR") or None
    res = run_bass_kernel_spmd(
        nc, in_maps, list(range(N_CORES)), trace=trace, tmpdir=tmpdir)
    LAST_RESULT = res

    loc = np.concatenate(
        [r["oloc"] for r in res.results], axis=0).astype(np.float32)
    glb = np.concatenate(
        [r["oglb"] for r in res.results], axis=0).astype(np.float32)
    # logit biases applied host-side (zero in this problem, kept general)
    blcat = np.concatenate(
        [np.asarray(b, np.float32) for b in (bl0, bl1, bl2, bl3)])
    if np.any(blcat):
        loc = loc + blcat
    bg_np = np.asarray(bg, np.float32)
    if np.any(bg_np):
        glb = glb + bg_np
    return loc, glb


# revision 26
# speedup vs baseline: 1.0640x; 1.0640x over previous
"""Trainium2 Bass kernel for a 4-layer LSTM-style stack with local+global logits.

Computation (per example row x of the [16384, 512] input):
    h0 = 0, c0 = 0
    for i in 1..4:
        z  = [x, h_{i-1}] @ W{f,i,o,c} + b        (4 gates, K = 1024)
        c  = tanh(z_c) * sigmoid(z_i) + sigmoid(z_f) * c
        h  = sigmoid(z_o) * tanh(c)
        local_i = h @ Wl_i + bl_i
    global = [x, h4] @ Wg + bg
Returns (concat(local_1..4) [16384, 960], global [16384, 960]).

Strategy:
  - Data-parallel over 8 cores: 2048 rows each, weights replicated.
  - The input half of the concat never changes: Z = x @ W_top + b is computed
    once per example and reused by all 4 layers; layer 1 needs no matmul at
    all (h0 = 0, gates = act(Z)).
  - Activations are kept transposed in SBUF (features on partitions, examples
    on the free dim) so gate matmuls need no transposes: the host supplies
    x already transposed.  Logits are computed in natural layout (examples on
    partitions) using H/X tiles as the stationary operand, so outputs DMA out
    without any transpose either.
  - bf16 matmul operands (1 cycle/row on the PE) and bf16 cell state /
    elementwise math (DVE tensor_tensor runs at 2x for bf16 SBUF operands).
  - Each core processes its 2048 rows as 4 quarters of 512 examples,
    software-pipelined two-at-a-time to keep the PE busy across the
    sequential layer boundaries.
"""

import os
import sys

import numpy as np

for _p in ("/opt/trn_rl_repo", "/root/.axon_site/_ro/trn_rl_repo"):
    if os.path.isdir(_p) and _p not in sys.path:
        sys.path.insert(0, _p)

import ml_dtypes

import concourse.bass as bass
import concourse.tile as tile
from concourse import bacc, mybir
from concourse.bass_utils import run_bass_kernel_spmd

BF16 = mybir.dt.bfloat16
F32 = mybir.dt.float32
FP8 = mybir.dt.float8e4
AF = mybir.ActivationFunctionType
ALU = mybir.AluOpType
DR = mybir.MatmulPerfMode.DoubleRow
WS = 128.0               # fp8 weight prescale / z storage scale

N_CORES = 8
N = 16384
K = 512                  # input features
U = 512                  # hidden units
GF = 4 * U               # 2048 concatenated gate features (order f, i, o, c)
MC = N // N_CORES        # 2048 rows per core
NQ = 4                   # quarters per core
EXQ = MC // NQ           # 512 examples per quarter
ET = EXQ // 128          # 4 example tiles of 128 per quarter
NCLS = [64, 128, 256, 512]
OFFS = [0, 64, 192, 448]
TOT = 960
N_LAYERS = 4

LAST_RESULT = None       # BassKernelResults of the most recent run (for test.py)


def _build_program():
    """Build the SPMD Bass program (identical on every core)."""
    nc = bacc.Bacc("TRN2", target_bir_lowering=False, debug=False)

    xt_d = nc.dram_tensor("xt", [K, MC], BF16, kind="ExternalInput")
    wcat_d = nc.dram_tensor("wcat", [K + U, GF], BF16, kind="ExternalInput")
    w8a_d = nc.dram_tensor("w8a", [128, 2, GF], FP8, kind="ExternalInput")
    w8b_d = nc.dram_tensor("w8b", [128, 2, GF], FP8, kind="ExternalInput")
    wl_d = nc.dram_tensor("wl", [U, TOT], BF16, kind="ExternalInput")
    wg_d = nc.dram_tensor("wg", [K + U, TOT], BF16, kind="ExternalInput")
    bgate_d = nc.dram_tensor("bgate", [128, 16], F32, kind="ExternalInput")
    blrep_d = nc.dram_tensor("blrep", [128, TOT], F32, kind="ExternalInput")
    bgrep_d = nc.dram_tensor("bgrep", [128, TOT], F32, kind="ExternalInput")
    oloc_d = nc.dram_tensor("oloc", [MC, TOT], F32, kind="ExternalOutput")
    oglb_d = nc.dram_tensor("oglb", [MC, TOT], F32, kind="ExternalOutput")

    with tile.TileContext(nc) as tc:
        with (
            tc.tile_pool(name="wpool", bufs=1) as wpool,
            tc.tile_pool(name="xpool", bufs=3) as xpool,
            tc.tile_pool(name="zpool", bufs=2) as zpool,
            tc.tile_pool(name="hpool", bufs=3) as hpool,
            tc.tile_pool(name="cpool", bufs=2) as cpool,
            tc.tile_pool(name="gatep", bufs=3) as gatep,
            tc.tile_pool(name="prep", bufs=6) as prep,
            tc.tile_pool(name="ttp", bufs=2) as ttp,
            tc.tile_pool(name="h8p", bufs=2) as h8p,
            tc.tile_pool(name="tcp", bufs=3) as tcp,
            tc.tile_pool(name="lop", bufs=4) as lop,
            tc.tile_pool(name="glop", bufs=2) as glop,
            tc.tile_pool(name="gpsum", bufs=8, space="PSUM") as gpsum,
        ):
            # ---- resident weights/biases --------------------------------
            # DMA emission order matters: the first Z-phase matmul needs only
            # x(q0) + the first 512-column group of W_top, so those bytes go
            # first (W_top is split into [128, 512] column tiles to keep the
            # dependency granularity small).
            xs_pre = {}
            tiles = []
            wtop_sb = [[None] * 4 for _ in range(4)]   # [kt][column group]
            for kt in range(4):
                t = xpool.tile([128, EXQ], BF16, tag=f"x{kt}")
                nc.sync.dma_start(t[:], xt_d[kt * 128:(kt + 1) * 128, 0:EXQ])
                tiles.append(t)
                w = wpool.tile([128, 512], BF16, tag=f"wt{kt}g0")
                nc.sync.dma_start(
                    w[:], wcat_d[kt * 128:(kt + 1) * 128, 0:512])
                wtop_sb[kt][0] = w
            xs_pre[0] = tiles
            bgate_sb = wpool.tile([128, 16], F32, tag="bgate")
            nc.sync.dma_start(bgate_sb[:], bgate_d[:])
            for g in range(1, 4):
                for kt in range(4):
                    t = wpool.tile([128, 512], BF16, tag=f"wt{kt}g{g}")
                    nc.sync.dma_start(
                        t[:], wcat_d[kt * 128:(kt + 1) * 128,
                                     g * 512:(g + 1) * 512])
                    wtop_sb[kt][g] = t
            tiles = []
            for kt in range(4):
                t = xpool.tile([128, EXQ], BF16, tag=f"x{kt}")
                nc.sync.dma_start(
                    t[:], xt_d[kt * 128:(kt + 1) * 128, EXQ:2 * EXQ])
                tiles.append(t)
            xs_pre[1] = tiles
            w8_sb = []
            for j, d in enumerate((w8a_d, w8b_d)):
                t = wpool.tile([128, 2, GF], FP8, tag=f"w8{j}")
                nc.sync.dma_start(t[:], d[:])
                w8_sb.append(t)
            wl_sb = []
            for kt in range(4):
                t = wpool.tile([128, TOT], BF16, tag=f"wl{kt}")
                nc.sync.dma_start(t[:], wl_d[kt * 128:(kt + 1) * 128, :])
                wl_sb.append(t)
            wg_sb = []
            for kt in range(8):
                t = wpool.tile([128, TOT], BF16, tag=f"wg{kt}")
                nc.sync.dma_start(t[:], wg_d[kt * 128:(kt + 1) * 128, :])
                wg_sb.append(t)
            blrep_sb = wpool.tile([128, TOT], F32, tag="blrep")
            nc.sync.dma_start(blrep_sb[:], blrep_d[:])
            bgrep_sb = wpool.tile([128, TOT], F32, tag="bgrep")
            nc.sync.dma_start(bgrep_sb[:], bgrep_d[:])

            # per-quarter live state
            xs = [None] * NQ      # 4 X^T tiles [128, EXQ] bf16
            zs = [None] * NQ      # 16 Z tiles [128, EXQ] bf16 (bias folded in)
            hs = [None] * NQ      # 4 H^T tiles [128, EXQ] bf16 (current layer)
            cs = [None] * NQ      # 4 C tiles [128, EXQ] bf16
            h8s = [None] * NQ     # 2 fp8 j-pair tiles [128, 2, EXQ]

            def stage_z(q):
                """DMA this quarter's x, compute Z = x @ W_top + b (bf16)."""
                if q in xs_pre:
                    xs[q] = xs_pre.pop(q)
                else:
                    xs[q] = []
                    for kt in range(4):
                        t = xpool.tile([128, EXQ], BF16, tag=f"x{kt}")
                        nc.sync.dma_start(
                            t[:], xt_d[kt * 128:(kt + 1) * 128,
                                       q * EXQ:(q + 1) * EXQ])
                        xs[q].append(t)
                zs[q] = []
                for of in range(16):
                    g, c = of // 4, of % 4
                    ps = gpsum.tile([128, EXQ], F32, tag="gp")
                    for kt in range(4):
                        nc.tensor.matmul(
                            ps[:], wtop_sb[kt][g][:, c * 128:(c + 1) * 128],
                            xs[q][kt][:], start=(kt == 0), stop=(kt == 3))
                    zt = zpool.tile([128, EXQ], BF16, tag=f"z{of}")
                    nc.vector.tensor_scalar(
                        zt[:], ps[:], WS, bgate_sb[:, of:of + 1],
                        ALU.mult, ALU.add)
                    zs[q].append(zt)

            def cand_update(q, t, ig, og, ch, fg, want_h8):
                """c = ch*ig (+ fg*c); h = og * tanh(c).  fg None on layer 1.
                Also casts h into the fp8 j-pair tile for the next layer's
                DoubleRow matmuls when want_h8."""
                if fg is None:
                    cn = cpool.tile([128, EXQ], BF16, tag=f"c{t}")
                    nc.vector.tensor_mul(cn[:], ig[:], ch[:])
                else:
                    t1 = ttp.tile([128, EXQ], BF16, tag="t1")
                    nc.vector.tensor_mul(t1[:], ig[:], ch[:])
                    t2 = ttp.tile([128, EXQ], BF16, tag="t2")
                    nc.vector.tensor_mul(t2[:], fg[:], cs[q][t][:])
                    cn = cpool.tile([128, EXQ], BF16, tag=f"c{t}")
                    nc.vector.tensor_add(cn[:], t1[:], t2[:])
                cs[q][t] = cn
                tc_t = tcp.tile([128, EXQ], BF16, tag="tc")
                nc.scalar.activation(tc_t[:], cn[:], AF.Tanh)
                if want_h8:
                    j, half = t // 2, t % 2
                    if half == 0:
                        h8 = h8p.tile([128, 2, EXQ], FP8, tag=f"h8{j}",
                                      name="h8")
                        h8s[q][j] = h8
                    nc.vector.tensor_mul(
                        h8s[q][j][:, half, :], og[:], tc_t[:])
                hn = hpool.tile([128, EXQ], BF16, tag=f"h{t}")
                nc.vector.tensor_mul(hn[:], og[:], tc_t[:])
                hs[q][t] = hn

            def stage_l1(q):
                """Layer 1: h0 = 0 so gates come straight from Z (no matmul)."""
                hs[q] = [None] * 4
                cs[q] = [None] * 4
                h8s[q] = [None, None]
                for t in range(4):
                    ig = gatep.tile([128, EXQ], BF16, tag="g1")
                    nc.scalar.activation(
                        ig[:], zs[q][4 + t][:], AF.Sigmoid, scale=1.0 / WS)
                    og = gatep.tile([128, EXQ], BF16, tag="g2")
                    nc.scalar.activation(
                        og[:], zs[q][8 + t][:], AF.Sigmoid, scale=1.0 / WS)
                    ch = gatep.tile([128, EXQ], BF16, tag="g3")
                    nc.scalar.activation(
                        ch[:], zs[q][12 + t][:], AF.Tanh, scale=1.0 / WS)
                    cand_update(q, t, ig, og, ch, None, True)

            def emit_locals(q, layer, h_tiles):
                """local_i = h_i @ Wl_i + bl_i, natural layout, DMA out."""
                off, ncl = OFFS[layer], NCLS[layer]
                for e in range(ET):
                    ps = gpsum.tile([128, 512], F32, tag="gp")
                    for kt in range(4):
                        nc.tensor.matmul(
                            ps[:, 0:ncl],
                            h_tiles[kt][:, e * 128:(e + 1) * 128],
                            wl_sb[kt][:, off:off + ncl],
                            start=(kt == 0), stop=(kt == 3))
                    ot = lop.tile([128, 512], F32, tag="lo")
                    nc.vector.tensor_add(
                        ot[:, 0:ncl], ps[:, 0:ncl], blrep_sb[:, off:off + ncl])
                    r0 = q * EXQ + e * 128
                    nc.sync.dma_start(
                        oloc_d[r0:r0 + 128, off:off + ncl], ot[:, 0:ncl])

            def emit_tblock(q, h8_prev, t, want_h8):
                """One hidtile's 4 gates + cand/h update for layers 2..4.
                Gate matmuls run in fp8 DoubleRow (K=256/instruction); all
                j=0 halves are emitted before the j=1 halves so the PE does
                not block on the previous layer's late h8[j1] cast."""
                pss = []
                for g in range(4):
                    ps = gpsum.tile([128, EXQ], F32, tag="gp", name="ps")
                    pss.append(ps)
                for j in range(2):
                    for g in range(4):
                        of = g * 4 + t
                        nc.tensor.matmul(
                            pss[g][:],
                            w8_sb[j][:, :, of * 128:(of + 1) * 128],
                            h8_prev[j][:], start=(j == 0), stop=(j == 1),
                            perf_mode=DR)
                gts = []
                for g in range(4):  # f, i, o, c
                    of = g * 4 + t
                    pre = prep.tile([128, EXQ], BF16, tag="pre")
                    nc.vector.tensor_tensor(
                        pre[:], pss[g][:], zs[q][of][:], ALU.add)
                    gt = gatep.tile([128, EXQ], BF16, tag=f"g{g}")
                    nc.scalar.activation(
                        gt[:], pre[:], AF.Tanh if g == 3 else AF.Sigmoid,
                        scale=1.0 / WS)
                    gts.append(gt)
                cand_update(q, t, gts[1], gts[2], gts[3], gts[0], want_h8)

            def stage_layer(q, layer):
                """Layers 2..4: gates = act(Z + h @ W_bot); then locals of the
                previous layer (ready at the same time, keeps the PE busy)."""
                h_prev = hs[q]
                h8_prev = h8s[q]
                hs[q] = [None] * 4
                h8s[q] = [None, None]
                emit_locals(q, layer - 2, h_prev)
                for t in range(4):
                    emit_tblock(q, h8_prev, t, layer < 4)

            def stage_layer2(qa, qb, layer):
                """Same layer for two quarters, hidtile-interleaved so one
                quarter's ready matmuls cover the other's DVE-chain waits."""
                hpa, hpb = hs[qa], hs[qb]
                h8a, h8b = h8s[qa], h8s[qb]
                hs[qa] = [None] * 4
                hs[qb] = [None] * 4
                h8s[qa] = [None, None]
                h8s[qb] = [None, None]
                emit_locals(qa, layer - 2, hpa)
                emit_locals(qb, layer - 2, hpb)
                for t in range(4):
                    emit_tblock(qa, h8a, t, layer < 4)
                    emit_tblock(qb, h8b, t, layer < 4)

            def stage_gl(q):
                """locals of layer 4, then global = [x, h4] @ Wg + bg."""
                emit_locals(q, 3, hs[q])
                xh = xs[q] + hs[q]
                for e in range(ET):
                    gt = glop.tile([128, TOT], F32, tag="glo")
                    for s0, s1 in ((0, 512), (512, TOT)):
                        ps = gpsum.tile([128, 512], F32, tag="gp")
                        w = s1 - s0
                        for kt in range(8):
                            nc.tensor.matmul(
                                ps[:, 0:w],
                                xh[kt][:, e * 128:(e + 1) * 128],
                                wg_sb[kt][:, s0:s1],
                                start=(kt == 0), stop=(kt == 7))
                        nc.vector.tensor_add(
                            gt[:, s0:s1], ps[:, 0:w], bgrep_sb[:, s0:s1])
                    r0 = q * EXQ + e * 128
                    nc.sync.dma_start(oglb_d[r0:r0 + 128, :], gt[:])

            # ---- software-pipelined emission (2 quarters in flight) -----
            plan = [
                (0, "Z"), (1, "Z"), (0, "L1"), (1, "L1"),
                (0, 2), (1, 2), (0, 3), (1, 3), (0, 4), (1, 4),
                (0, "GL"), (2, "Z"), (2, "L1"), (1, "GL"),
                (3, "Z"), (3, "L1"),
                (2, 2), (3, 2), (2, 3), (3, 3), (2, 4), (3, 4),
                (2, "GL"), (3, "GL"),
            ]
            for q, s in plan:
                if s == "Z":
                    stage_z(q)
                elif s == "L1":
                    stage_l1(q)
                elif s == "GL":
                    stage_gl(q)
                elif isinstance(q, tuple):
                    stage_layer2(q[0], q[1], s)
                else:
                    stage_layer(q, s)

    nc.compile()
    return nc


_PROGRAM = None


def _get_program():
    global _PROGRAM
    if _PROGRAM is None:
        _PROGRAM = _build_program()
    return _PROGRAM


def kernel(inputs, Wf, bf, Wi, bi, Wo, bo, Wc, bc,
           Wl0, bl0, Wl1, bl1, Wl2, bl2, Wl3, bl3, Wg, bg):
    global LAST_RESULT
    bf16 = ml_dtypes.bfloat16

    inputs = np.ascontiguousarray(np.asarray(inputs, dtype=np.float32))
    xt_all = inputs.T.astype(bf16)                    # [512, 16384]
    fp8 = ml_dtypes.float8_e4m3
    wcat_f = np.concatenate(
        [np.asarray(w, np.float32) for w in (Wf, Wi, Wo, Wc)], axis=1)
    wcat = wcat_f.astype(bf16)                        # [1024, 2048]
    wbot = wcat_f[512:]                               # [512, 2048]
    w8 = []
    for j in range(2):
        a = np.empty((128, 2, 2048), np.float32)
        for i in range(2):
            a[:, i, :] = wbot[128 * (2 * j + i):128 * (2 * j + i + 1), :]
        w8.append(np.ascontiguousarray((a * 128.0).astype(fp8)))
    bcat = np.concatenate(
        [np.asarray(b, np.float32) for b in (bf, bi, bo, bc)])  # [2048]
    bgate = np.ascontiguousarray(
        128.0 * bcat.reshape(16, 128).T)                        # [128, 16]
    wl = np.concatenate(
        [np.asarray(w, np.float32) for w in (Wl0, Wl1, Wl2, Wl3)],
        axis=1).astype(bf16)                          # [512, 960]
    blrep = np.ascontiguousarray(np.broadcast_to(
        np.concatenate([np.asarray(b, np.float32)
                        for b in (bl0, bl1, bl2, bl3)]), (128, TOT)))
    wg = np.asarray(Wg, np.float32).astype(bf16)      # [1024, 960]
    bgrep = np.ascontiguousarray(
        np.broadcast_to(np.asarray(bg, np.float32), (128, TOT)))

    in_maps = []
    for c in range(N_CORES):
        in_maps.append({
            "xt": np.ascontiguousarray(xt_all[:, c * MC:(c + 1) * MC]),
            "wcat": wcat, "w8a": w8[0], "w8b": w8[1], "wl": wl, "wg": wg,
            "bgate": bgate, "blrep": blrep, "bgrep": bgrep,
        })

    nc = _get_program()
    trace = os.environ.get("BASS_KERNEL_TRACE", "0") == "1"
    tmpdir = os.environ.get("BASS_KERNEL_TMPDIR") or None
    res = run_bass_kernel_spmd(
        nc, in_maps, list(range(N_CORES)), trace=trace, tmpdir=tmpdir)
    LAST_RESULT = res

    loc = np.concatenate([r["oloc"] for r in res.results], axis=0)
    glb = np.concatenate([r["oglb"] for r in res.results], axis=0)
    return loc, glb



# revision 28
# speedup vs baseline: 1.0967x; 1.0308x over previous
"""Trainium2 Bass kernel for a 4-layer LSTM-style stack with local+global logits.

Computation (per example row x of the [16384, 512] input):
    h0 = 0, c0 = 0
    for i in 1..4:
        z  = [x, h_{i-1}] @ W{f,i,o,c} + b        (4 gates, K = 1024)
        c  = tanh(z_c) * sigmoid(z_i) + sigmoid(z_f) * c
        h  = sigmoid(z_o) * tanh(c)
        local_i = h @ Wl_i + bl_i
    global = [x, h4] @ Wg + bg
Returns (concat(local_1..4) [16384, 960], global [16384, 960]).

Strategy:
  - Data-parallel over 8 cores: 2048 rows each, weights replicated.
  - The input half of the concat never changes: Z = x @ W_top + b is computed
    once per example and reused by all 4 layers; layer 1 needs no matmul at
    all (h0 = 0, gates = act(Z)).
  - Activations are kept transposed in SBUF (features on partitions, examples
    on the free dim) so gate matmuls need no transposes: the host supplies
    x already transposed.  Logits are computed in natural layout (examples on
    partitions) using H/X tiles as the stationary operand, so outputs DMA out
    without any transpose either.
  - bf16 matmul operands (1 cycle/row on the PE) and bf16 cell state /
    elementwise math (DVE tensor_tensor runs at 2x for bf16 SBUF operands).
  - Each core processes its 2048 rows as 4 quarters of 512 examples,
    software-pipelined two-at-a-time to keep the PE busy across the
    sequential layer boundaries.
"""

import os
import sys

import numpy as np

for _p in ("/opt/trn_rl_repo", "/root/.axon_site/_ro/trn_rl_repo"):
    if os.path.isdir(_p) and _p not in sys.path:
        sys.path.insert(0, _p)

import ml_dtypes

import concourse.bass as bass
import concourse.tile as tile
from concourse import bacc, mybir
from concourse.bass_utils import run_bass_kernel_spmd

BF16 = mybir.dt.bfloat16
F32 = mybir.dt.float32
FP8 = mybir.dt.float8e4
AF = mybir.ActivationFunctionType
ALU = mybir.AluOpType
DR = mybir.MatmulPerfMode.DoubleRow
WS = 128.0               # fp8 weight prescale / z storage scale

N_CORES = 8
N = 16384
K = 512                  # input features
U = 512                  # hidden units
GF = 4 * U               # 2048 concatenated gate features (order f, i, o, c)
MC = N // N_CORES        # 2048 rows per core
NQ = 4                   # quarters per core
EXQ = MC // NQ           # 512 examples per quarter
ET = EXQ // 128          # 4 example tiles of 128 per quarter
NCLS = [64, 128, 256, 512]
OFFS = [0, 64, 192, 448]
TOT = 960
N_LAYERS = 4

LAST_RESULT = None       # BassKernelResults of the most recent run (for test.py)


def _build_program():
    """Build the SPMD Bass program (identical on every core)."""
    nc = bacc.Bacc("TRN2", target_bir_lowering=False, debug=False)

    xt_d = nc.dram_tensor("xt", [K, MC], BF16, kind="ExternalInput")
    wcat_d = nc.dram_tensor("wcat", [K + U, GF], BF16, kind="ExternalInput")
    w8a_d = nc.dram_tensor("w8a", [128, 2, GF], FP8, kind="ExternalInput")
    w8b_d = nc.dram_tensor("w8b", [128, 2, GF], FP8, kind="ExternalInput")
    wl_d = nc.dram_tensor("wl", [U, TOT], BF16, kind="ExternalInput")
    wg_d = nc.dram_tensor("wg", [K + U, TOT], BF16, kind="ExternalInput")
    bgate_d = nc.dram_tensor("bgate", [128, 16], F32, kind="ExternalInput")
    blrep_d = nc.dram_tensor("blrep", [128, TOT], F32, kind="ExternalInput")
    bgrep_d = nc.dram_tensor("bgrep", [128, TOT], F32, kind="ExternalInput")
    oloc_d = nc.dram_tensor("oloc", [MC, TOT], F32, kind="ExternalOutput")
    oglb_d = nc.dram_tensor("oglb", [MC, TOT], F32, kind="ExternalOutput")

    with tile.TileContext(nc) as tc:
        with (
            tc.tile_pool(name="wpool", bufs=1) as wpool,
            tc.tile_pool(name="xpool", bufs=3) as xpool,
            tc.tile_pool(name="zpool", bufs=2) as zpool,
            tc.tile_pool(name="hpool", bufs=3) as hpool,
            tc.tile_pool(name="cpool", bufs=2) as cpool,
            tc.tile_pool(name="gatep", bufs=3) as gatep,
            tc.tile_pool(name="prep", bufs=6) as prep,
            tc.tile_pool(name="ttp", bufs=2) as ttp,
            tc.tile_pool(name="h8p", bufs=2) as h8p,
            tc.tile_pool(name="tcp", bufs=3) as tcp,
            tc.tile_pool(name="lop", bufs=4) as lop,
            tc.tile_pool(name="glop", bufs=2) as glop,
            tc.tile_pool(name="gpsum", bufs=8, space="PSUM") as gpsum,
        ):
            # ---- resident weights/biases --------------------------------
            # DMA emission order matters: the first Z-phase matmul needs only
            # x(q0) + the first 512-column group of W_top, so those bytes go
            # first (W_top is split into [128, 512] column tiles to keep the
            # dependency granularity small).
            xs_pre = {}
            tiles = []
            wtop_sb = [[None] * 4 for _ in range(4)]   # [kt][column group]
            for kt in range(4):
                t = xpool.tile([128, EXQ], BF16, tag=f"x{kt}")
                nc.sync.dma_start(t[:], xt_d[kt * 128:(kt + 1) * 128, 0:EXQ])
                tiles.append(t)
                w = wpool.tile([128, 512], BF16, tag=f"wt{kt}g0")
                nc.sync.dma_start(
                    w[:], wcat_d[kt * 128:(kt + 1) * 128, 0:512])
                wtop_sb[kt][0] = w
            xs_pre[0] = tiles
            bgate_sb = wpool.tile([128, 16], F32, tag="bgate")
            nc.sync.dma_start(bgate_sb[:], bgate_d[:])
            for g in range(1, 4):
                for kt in range(4):
                    t = wpool.tile([128, 512], BF16, tag=f"wt{kt}g{g}")
                    nc.sync.dma_start(
                        t[:], wcat_d[kt * 128:(kt + 1) * 128,
                                     g * 512:(g + 1) * 512])
                    wtop_sb[kt][g] = t
            tiles = []
            for kt in range(4):
                t = xpool.tile([128, EXQ], BF16, tag=f"x{kt}")
                nc.sync.dma_start(
                    t[:], xt_d[kt * 128:(kt + 1) * 128, EXQ:2 * EXQ])
                tiles.append(t)
            xs_pre[1] = tiles
            w8_sb = []
            for j, d in enumerate((w8a_d, w8b_d)):
                t = wpool.tile([128, 2, GF], FP8, tag=f"w8{j}")
                nc.sync.dma_start(t[:], d[:])
                w8_sb.append(t)
            wl_sb = []
            for kt in range(4):
                t = wpool.tile([128, TOT], BF16, tag=f"wl{kt}")
                nc.sync.dma_start(t[:], wl_d[kt * 128:(kt + 1) * 128, :])
                wl_sb.append(t)
            wg_sb = []
            for kt in range(8):
                t = wpool.tile([128, TOT], BF16, tag=f"wg{kt}")
                nc.sync.dma_start(t[:], wg_d[kt * 128:(kt + 1) * 128, :])
                wg_sb.append(t)
            blrep_sb = wpool.tile([128, TOT], F32, tag="blrep")
            nc.sync.dma_start(blrep_sb[:], blrep_d[:])
            bgrep_sb = wpool.tile([128, TOT], F32, tag="bgrep")
            nc.sync.dma_start(bgrep_sb[:], bgrep_d[:])

            # per-quarter live state
            xs = [None] * NQ      # 4 X^T tiles [128, EXQ] bf16
            zs = [None] * NQ      # 16 Z tiles [128, EXQ] bf16 (bias folded in)
            hs = [None] * NQ      # 4 H^T tiles [128, EXQ] bf16 (current layer)
            cs = [None] * NQ      # 4 C tiles [128, EXQ] bf16
            h8s = [None] * NQ     # 2 fp8 j-pair tiles [128, 2, EXQ]

            def stage_z(q):
                """DMA this quarter's x, compute Z = x @ W_top + b (bf16)."""
                if q in xs_pre:
                    xs[q] = xs_pre.pop(q)
                else:
                    xs[q] = []
                    for kt in range(4):
                        t = xpool.tile([128, EXQ], BF16, tag=f"x{kt}")
                        nc.sync.dma_start(
                            t[:], xt_d[kt * 128:(kt + 1) * 128,
                                       q * EXQ:(q + 1) * EXQ])
                        xs[q].append(t)
                zs[q] = []
                for of in range(16):
                    g, c = of // 4, of % 4
                    ps = gpsum.tile([128, EXQ], F32, tag="gp")
                    for kt in range(4):
                        nc.tensor.matmul(
                            ps[:], wtop_sb[kt][g][:, c * 128:(c + 1) * 128],
                            xs[q][kt][:], start=(kt == 0), stop=(kt == 3))
                    zt = zpool.tile([128, EXQ], BF16, tag=f"z{of}")
                    if of % 2 == 0:
                        nc.vector.tensor_scalar(
                            zt[:], ps[:], WS, bgate_sb[:, of:of + 1],
                            ALU.mult, ALU.add)
                    else:
                        nc.scalar.activation(
                            zt[:], ps[:], AF.Identity,
                            bias=bgate_sb[:, of:of + 1], scale=WS)
                    zs[q].append(zt)

            def cand_update(q, t, ig, og, ch, fg, want_h8):
                """c = ch*ig (+ fg*c); h = og * tanh(c).  fg None on layer 1.
                Also casts h into the fp8 j-pair tile for the next layer's
                DoubleRow matmuls when want_h8."""
                if fg is None:
                    cn = cpool.tile([128, EXQ], BF16, tag=f"c{t}")
                    nc.vector.tensor_mul(cn[:], ig[:], ch[:])
                else:
                    t1 = ttp.tile([128, EXQ], BF16, tag="t1")
                    nc.vector.tensor_mul(t1[:], ig[:], ch[:])
                    t2 = ttp.tile([128, EXQ], BF16, tag="t2")
                    nc.vector.tensor_mul(t2[:], fg[:], cs[q][t][:])
                    cn = cpool.tile([128, EXQ], BF16, tag=f"c{t}")
                    nc.vector.tensor_add(cn[:], t1[:], t2[:])
                cs[q][t] = cn
                tc_t = tcp.tile([128, EXQ], BF16, tag="tc")
                nc.scalar.activation(tc_t[:], cn[:], AF.Tanh)
                if want_h8:
                    j, half = t // 2, t % 2
                    if half == 0:
                        h8 = h8p.tile([128, 2, EXQ], FP8, tag=f"h8{j}",
                                      name="h8")
                        h8s[q][j] = h8
                    nc.vector.tensor_mul(
                        h8s[q][j][:, half, :], og[:], tc_t[:])
                hn = hpool.tile([128, EXQ], BF16, tag=f"h{t}")
                nc.vector.tensor_mul(hn[:], og[:], tc_t[:])
                hs[q][t] = hn

            def stage_l1(q):
                """Layer 1: h0 = 0 so gates come straight from Z (no matmul)."""
                hs[q] = [None] * 4
                cs[q] = [None] * 4
                h8s[q] = [None, None]
                for t in range(4):
                    ig = gatep.tile([128, EXQ], BF16, tag="g1")
                    nc.scalar.activation(
                        ig[:], zs[q][4 + t][:], AF.Sigmoid, scale=1.0 / WS)
                    og = gatep.tile([128, EXQ], BF16, tag="g2")
                    nc.scalar.activation(
                        og[:], zs[q][8 + t][:], AF.Sigmoid, scale=1.0 / WS)
                    ch = gatep.tile([128, EXQ], BF16, tag="g3")
                    nc.scalar.activation(
                        ch[:], zs[q][12 + t][:], AF.Tanh, scale=1.0 / WS)
                    cand_update(q, t, ig, og, ch, None, True)

            def emit_locals(q, layer, h_tiles):
                """local_i = h_i @ Wl_i + bl_i, natural layout, DMA out."""
                off, ncl = OFFS[layer], NCLS[layer]
                for e in range(ET):
                    ps = gpsum.tile([128, 512], F32, tag="gp")
                    for kt in range(4):
                        nc.tensor.matmul(
                            ps[:, 0:ncl],
                            h_tiles[kt][:, e * 128:(e + 1) * 128],
                            wl_sb[kt][:, off:off + ncl],
                            start=(kt == 0), stop=(kt == 3))
                    ot = lop.tile([128, 512], F32, tag="lo")
                    nc.scalar.activation(
                        ot[:, 0:ncl], ps[:, 0:ncl], AF.Copy)
                    r0 = q * EXQ + e * 128
                    nc.sync.dma_start(
                        oloc_d[r0:r0 + 128, off:off + ncl], ot[:, 0:ncl])

            def emit_tblock(q, h8_prev, t, want_h8):
                """One hidtile's 4 gates + cand/h update for layers 2..4.
                Gate matmuls run in fp8 DoubleRow (K=256/instruction); all
                j=0 halves are emitted before the j=1 halves so the PE does
                not block on the previous layer's late h8[j1] cast."""
                pss = []
                for g in range(4):
                    ps = gpsum.tile([128, EXQ], F32, tag="gp", name="ps")
                    pss.append(ps)
                for j in range(2):
                    for g in range(4):
                        of = g * 4 + t
                        nc.tensor.matmul(
                            pss[g][:],
                            w8_sb[j][:, :, of * 128:(of + 1) * 128],
                            h8_prev[j][:], start=(j == 0), stop=(j == 1),
                            perf_mode=DR)
                gts = []
                for g in range(4):  # f, i, o, c
                    of = g * 4 + t
                    pre = prep.tile([128, EXQ], BF16, tag="pre")
                    nc.vector.tensor_tensor(
                        pre[:], pss[g][:], zs[q][of][:], ALU.add)
                    gt = gatep.tile([128, EXQ], BF16, tag=f"g{g}")
                    nc.scalar.activation(
                        gt[:], pre[:], AF.Tanh if g == 3 else AF.Sigmoid,
                        scale=1.0 / WS)
                    gts.append(gt)
                cand_update(q, t, gts[1], gts[2], gts[3], gts[0], want_h8)

            def stage_layer(q, layer):
                """Layers 2..4: gates = act(Z + h @ W_bot); then locals of the
                previous layer (ready at the same time, keeps the PE busy)."""
                h_prev = hs[q]
                h8_prev = h8s[q]
                hs[q] = [None] * 4
                h8s[q] = [None, None]
                emit_locals(q, layer - 2, h_prev)
                for t in range(4):
                    emit_tblock(q, h8_prev, t, layer < 4)

            def stage_layer2(qa, qb, layer):
                """Same layer for two quarters, hidtile-interleaved so one
                quarter's ready matmuls cover the other's DVE-chain waits."""
                hpa, hpb = hs[qa], hs[qb]
                h8a, h8b = h8s[qa], h8s[qb]
                hs[qa] = [None] * 4
                hs[qb] = [None] * 4
                h8s[qa] = [None, None]
                h8s[qb] = [None, None]
                emit_locals(qa, layer - 2, hpa)
                emit_locals(qb, layer - 2, hpb)
                for t in range(4):
                    emit_tblock(qa, h8a, t, layer < 4)
                    emit_tblock(qb, h8b, t, layer < 4)

            def stage_gl(q):
                """locals of layer 4, then global = [x, h4] @ Wg + bg."""
                emit_locals(q, 3, hs[q])
                xh = xs[q] + hs[q]
                for e in range(ET):
                    gt = glop.tile([128, TOT], F32, tag="glo")
                    for s0, s1 in ((0, 512), (512, TOT)):
                        ps = gpsum.tile([128, 512], F32, tag="gp")
                        w = s1 - s0
                        for kt in range(8):
                            nc.tensor.matmul(
                                ps[:, 0:w],
                                xh[kt][:, e * 128:(e + 1) * 128],
                                wg_sb[kt][:, s0:s1],
                                start=(kt == 0), stop=(kt == 7))
                        nc.scalar.activation(
                            gt[:, s0:s1], ps[:, 0:w], AF.Copy)
                    r0 = q * EXQ + e * 128
                    nc.sync.dma_start(oglb_d[r0:r0 + 128, :], gt[:])

            # ---- software-pipelined emission (2 quarters in flight) -----
            plan = [
                (0, "Z"), (1, "Z"), (0, "L1"), (1, "L1"),
                (0, 2), (1, 2), (0, 3), (1, 3), (0, 4), (1, 4),
                (0, "GL"), (2, "Z"), (2, "L1"), (1, "GL"),
                (3, "Z"), (3, "L1"),
                (2, 2), (3, 2), (2, 3), (3, 3), (2, 4), (3, 4),
                (2, "GL"), (3, "GL"),
            ]
            for q, s in plan:
                if s == "Z":
                    stage_z(q)
                elif s == "L1":
                    stage_l1(q)
                elif s == "GL":
                    stage_gl(q)
                elif isinstance(q, tuple):
                    stage_layer2(q[0], q[1], s)
                else:
                    stage_layer(q, s)

    nc.compile()
    return nc


_PROGRAM = None


def _get_program():
    global _PROGRAM
    if _PROGRAM is None:
        _PROGRAM = _build_program()
    return _PROGRAM


def kernel(inputs, Wf, bf, Wi, bi, Wo, bo, Wc, bc,
           Wl0, bl0, Wl1, bl1, Wl2, bl2, Wl3, bl3, Wg, bg):
    global LAST_RESULT
    bf16 = ml_dtypes.bfloat16

    inputs = np.ascontiguousarray(np.asarray(inputs, dtype=np.float32))
    xt_all = inputs.T.astype(bf16)                    # [512, 16384]
    fp8 = ml_dtypes.float8_e4m3
    wcat_f = np.concatenate(
        [np.asarray(w, np.float32) for w in (Wf, Wi, Wo, Wc)], axis=1)
    wcat = wcat_f.astype(bf16)                        # [1024, 2048]
    wbot = wcat_f[512:]                               # [512, 2048]
    w8 = []
    for j in range(2):
        a = np.empty((128, 2, 2048), np.float32)
        for i in range(2):
            a[:, i, :] = wbot[128 * (2 * j + i):128 * (2 * j + i + 1), :]
        w8.append(np.ascontiguousarray((a * 128.0).astype(fp8)))
    bcat = np.concatenate(
        [np.asarray(b, np.float32) for b in (bf, bi, bo, bc)])  # [2048]
    bgate = np.ascontiguousarray(
        128.0 * bcat.reshape(16, 128).T)                        # [128, 16]
    wl = np.concatenate(
        [np.asarray(w, np.float32) for w in (Wl0, Wl1, Wl2, Wl3)],
        axis=1).astype(bf16)                          # [512, 960]
    blrep = np.ascontiguousarray(np.broadcast_to(
        np.concatenate([np.asarray(b, np.float32)
                        for b in (bl0, bl1, bl2, bl3)]), (128, TOT)))
    wg = np.asarray(Wg, np.float32).astype(bf16)      # [1024, 960]
    bgrep = np.ascontiguousarray(
        np.broadcast_to(np.asarray(bg, np.float32), (128, TOT)))

    in_maps = []
    for c in range(N_CORES):
        in_maps.append({
            "xt": np.ascontiguousarray(xt_all[:, c * MC:(c + 1) * MC]),
            "wcat": wcat, "w8a": w8[0], "w8b": w8[1], "wl": wl, "wg": wg,
            "bgate": bgate, "blrep": blrep, "bgrep": bgrep,
        })

    nc = _get_program()
    trace = os.environ.get("BASS_KERNEL_TRACE", "0") == "1"
    tmpdir = os.environ.get("BASS_KERNEL_TMPDIR") or None
    res = run_bass_kernel_spmd(
        nc, in_maps, list(range(N_CORES)), trace=trace, tmpdir=tmpdir)
    LAST_RESULT = res

    loc = np.concatenate([r["oloc"] for r in res.results], axis=0)
    glb = np.concatenate([r["oglb"] for r in res.results], axis=0)
    blcat = np.concatenate(
        [np.asarray(b, np.float32) for b in (bl0, bl1, bl2, bl3)])
    if np.any(blcat):
        loc = loc + blcat
    bg_np = np.asarray(bg, np.float32)
    if np.any(bg_np):
        glb = glb + bg_np
    return loc, glb

